# revision 1
# baseline (speedup 1.0000x reference)
"""Trainium2 Bass kernel for CSSM (Mamba-style 2D selective scan block).

Sharding: 8 cores = 4 batch x 2 d_inner-halves. Each core computes the
full front-end (convs/projections) for its batch element, the selective
scan for its 96 d_inner channels x 16 states, and a partial output
projection. The host sums the two partial outputs per batch element.

Scan layout: one (96, L) tile per state index n; the recurrence
h_t = dA_t*h_{t-1} + dBu_t maps onto the native tensor_tensor_scan
instruction (channels on partitions, time along the free dim), split
across the Vector and GPSIMD engines. dA_n = exp(-a[d,n]*delta) comes
straight from the scalar engine using a per-partition scale operand.
B/C rows are broadcast to all channel partitions by DMA from a DRAM
staging copy of x_proj's output. y = sum_n C.h is accumulated on the
tensor engine via an identity matmul into PSUM.
"""
import sys

sys.path.insert(0, "/opt/trn_rl_repo")

import numpy as np

C = 96            # d_model; also channels per d_inner half
DI = 192          # d_inner
NST = 16          # d_state
DTR = 6           # dt_rank
HH = 64
WW = 64
L = HH * WW       # 4096
T = 512           # matmul moving-dim chunk
T2 = 1024         # scan-phase chunk (2 matmul chunks)
NB = L // T2      # 4 scan-phase blocks
PW = WW + 2       # 66: padded row width for the 3x3 conv
G = 68            # left guard of the padded conv buffer
PADLEN = G + (HH + 2) * PW + 68
NGP = 16          # states n >= NGP: dBu multiplies on GPSIMD (16 = none)
# (col_start, width) pipeline blocks; small leading blocks cut the fill
BLOCKS = ((0, 1024), (1024, 1024), (2048, 1024), (3072, 1024))
TMP_GP = frozenset()  # tmp products routed to GPSIMD

_CACHE = {}


def _emit(tc, nc, mybir, dram):
    from contextlib import ExitStack

    from concourse import bass

    f32 = mybir.dt.float32
    bf16 = mybir.dt.bfloat16
    AF = mybir.ActivationFunctionType
    OP = mybir.AluOpType

    def mmacc(out, pairs, start=True, stop=True, ncols=None):
        """Matmul with free-dim split into even <=512-col PSUM-bank groups."""
        n = ncols if ncols is not None else out.shape[-1]
        nchunk = -(-n // T)
        step = -(-n // nchunk)
        bounds = list(range(0, n, step)) + [n]
        for c0, c1 in zip(bounds[:-1], bounds[1:]):
            for i, (lh, rh) in enumerate(pairs):
                nc.tensor.matmul(out[:, c0:c1], lh, rh[:, c0:c1],
                                 start=start and i == 0,
                                 stop=stop and i == len(pairs) - 1)

    with ExitStack() as ctx:
        ec = ctx.enter_context
        consts = ec(tc.tile_pool(name="consts", bufs=1))
        persist = ec(tc.tile_pool(name="persist", bufs=1))
        dpool = ec(tc.tile_pool(name="dpool", bufs=1, space="DRAM"))
        fw = ec(tc.tile_pool(name="fw", bufs=1))
        pxck = ec(tc.tile_pool(name="pxck", bufs=2))
        pxc2 = ec(tc.tile_pool(name="pxc2", bufs=2))
        pxin = ec(tc.tile_pool(name="pxin", bufs=4))
        pxca = ec(tc.tile_pool(name="pxca", bufs=3))
        pxcb = ec(tc.tile_pool(name="pxcb", bufs=2))
        psz = ec(tc.tile_pool(name="psz", bufs=2))
        pxd = ec(tc.tile_pool(name="pxd", bufs=2))
        dl = ec(tc.tile_pool(name="dl", bufs=2))
        lp = ec(tc.tile_pool(name="lp", bufs=3))
        hp = ec(tc.tile_pool(name="hp", bufs=3))
        bc = ec(tc.tile_pool(name="bc", bufs=18))
        tl = ec(tc.tile_pool(name="tl", bufs=2))
        fps = ec(tc.tile_pool(name="fps", bufs=2, space="PSUM"))
        psy = ec(tc.tile_pool(name="psy", bufs=2, space="PSUM"))
        pmix = ec(tc.tile_pool(name="pmix", bufs=2, space="PSUM"))

        def cload(name, shape, dtype=f32, rearr=None, pool=None):
            t = (pool or consts).tile(list(shape), dtype, tag=name)
            src = dram[name]
            if rearr is not None:
                src = src.rearrange(rearr)
            nc.sync.dma_start(t[:], src)
            return t

        wdt_sb = cload("wdt", (32, C), bf16)
        bdt_sb = cload("bdt", (C, 1))
        dvec_sb = cload("dvec", (C, 1))
        wout_sb = cload("wout", (C, C), bf16)
        b1d_sb = cload("b1d", (C, 2))
        aneg_sb = cload("aneg", (C, NST))       # -exp(A_log), local rows
        ident_sb = cload("ident", (C, C), bf16)
        wp_sb = cload("wp", (C, DI), bf16, pool=fw)
        wd_sb = cload("wd", (C, 18, C), bf16, "t g k m -> k (t g) m", pool=fw)
        win_sb = cload("win", (C, 3 * C), bf16, pool=fw)
        w1d_sb = cload("w1d", (C, 8, C), bf16, "g t k m -> k (g t) m", pool=fw)
        wxp_sb = cload("wxp", (C, 2, 38), bf16, "g k m -> k g m", pool=fw)
        # note: wxp loaded as (96, 2, 38) via the rearrange

        carry = persist.tile([C, NST], f32, tag="carry")
        xdd = dpool.tile([38, L], bf16, tag="xdd")

        xp1 = [persist.tile([C, PADLEN], bf16, tag=f"xp1_{g}", name=f"xp1_{g}")
               for g in range(2)]
        nc.gpsimd.memset(xp1[0][:], 0.0)
        nc.gpsimd.memset(xp1[1][:], 0.0)

        xinp_prev = [None, None]

        state = {"xinp_prev": [None, None]}

        def _front(s):
            cs, bw = BLOCKS[s]
            ce = cs + bw
            nhh = bw // T
            # ---- 3x3 dconv (<=7-row units; one PSUM bank each) -> xc2 ----
            nrow = bw // WW
            units = []
            ro = 0
            while ro < nrow:
                units.append((ro, min(7, nrow - ro)))
                ro += 7
            xc2 = pxc2.tile([C, T2], bf16, tag="xc2", name=f"xc2_{s}")[:, :bw]
            for u, (ro, rows) in enumerate(units):
                r0 = cs // WW + ro
                cols = rows * PW
                base = G + (r0 + 1) * PW
                psd = fps.tile([C, T2], f32, tag="fps", name=f"dconv_{s}_{u}")
                pairs = []
                for tap in range(9):
                    dy, dx = tap // 3, tap % 3
                    shift = (dy - 1) * PW + (dx - 1)
                    for g in range(2):
                        pairs.append((wd_sb[:, tap * 2 + g, :],
                                      xp1[g][:, base + shift: base + shift + cols]))
                mmacc(psd[:, :cols], pairs, ncols=cols)
                srcv = psd[:, :cols].rearrange("p (r w) -> p r w", w=PW)[:, :, 1:65]
                dstv = xc2[:, ro * WW: (ro + rows) * WW]
                nc.scalar.activation(dstv.rearrange("p (r w) -> p r w", w=WW), srcv,
                                     AF.Copy)

            # ---- in_proj -> xinp halves + silu(z) ----
            xinp = [pxin.tile([C, 3 + T2], bf16, tag="xinp",
                              name=f"xinp_{s}_{g}")[:, :3 + bw]
                    for g in range(2)]
            sz = psz.tile([C, T2], bf16, tag="sz", name=f"sz_{s}")[:, :bw]
            for g in range(2):
                if s == 0:
                    nc.gpsimd.memset(xinp[g][:, 0:3], 0.0)
                else:
                    pw = BLOCKS[s - 1][1]
                    nc.gpsimd.tensor_copy(xinp[g][:, 0:3],
                                          state["xinp_prev"][g][:, pw:pw + 3])
            for g in range(3):
                ps = fps.tile([C, T2], f32, tag="fps", name=f"inp_{s}_{g}")[:, :bw]
                mmacc(ps, [(win_sb[:, g * C:(g + 1) * C], xc2[:])])
                if g < 2:
                    nc.scalar.activation(xinp[g][:, 3:3 + bw], ps[:], AF.Copy)
                else:
                    nc.scalar.activation(sz[:], ps[:], AF.Silu)
            state["xinp_prev"] = xinp

            # ---- causal depthwise conv1d + silu -> xc halves ----
            xc_a = pxca.tile([C, T2], bf16, tag="xc_a", name=f"xc_a_{s}")[:, :bw]
            xc_b = pxcb.tile([C, T2], bf16, tag="xc_b", name=f"xc_b_{s}")[:, :bw]
            for g in range(2):
                ps = fps.tile([C, T2], f32, tag="fps", name=f"c1d_{s}_{g}")[:, :bw]
                mmacc(ps, [(w1d_sb[:, g * 4 + k, :], xinp[g][:, k:k + bw])
                           for k in range(4)])
                nc.scalar.activation(xc_a[:] if g == 0 else xc_b[:], ps[:],
                                     AF.Silu, bias=b1d_sb[:, g:g + 1])

            # ---- x_proj -> x_dbl block, staged to DRAM ----
            psx = fps.tile([38, T2], f32, tag="fps", name=f"xp_{s}")[:, :bw]
            mmacc(psx, [(wxp_sb[:, 0, :], xc_a[:]), (wxp_sb[:, 1, :], xc_b[:])])
            x_dbl = pxd.tile([38, T2], bf16, tag="x_dbl", name=f"x_dbl_{s}")[:, :bw]
            nc.scalar.activation(x_dbl[:], psx[:], AF.Copy)
            nc.sync.dma_start(xdd[:, cs:ce], x_dbl[:])

            # ---- B/C broadcasts + delta/du: ready before the scan stage ----
            bbcc = []
            for n in range(NST):
                t = bc.tile([C, 2, T2], bf16, tag="bc", name=f"bc_{s}_{n}")[:, :, :bw]
                row = xdd[DTR + n: DTR + n + 1, cs:ce]
                srcb = bass.AP(tensor=row.tensor, offset=row.offset,
                               ap=[[0, C], [NST * L, 2], [1, bw]])
                nc.sync.dma_start(t[:], srcb)
                bbcc.append(t)

            edt = dl.tile([C, T2], f32, tag="edt", name=f"edt_{s}")[:, :bw]
            delta_c = dl.tile([C, T2], f32, tag="delta", name=f"delta_{s}")[:, :bw]
            for hh in range(nhh):
                psD = pmix.tile([C, T], f32, tag="pmix", name=f"psD_{s}_{hh}")
                nc.tensor.matmul(psD[:], wdt_sb[:],
                                 x_dbl[:32, hh * T:(hh + 1) * T])
                nc.scalar.activation(edt[:, hh * T:(hh + 1) * T], psD[:],
                                     AF.Exp, bias=bdt_sb[:])
            nc.scalar.activation(delta_c[:], edt[:], AF.Ln, bias=1.0)
            du_c = dl.tile([C, T2], bf16, tag="du", name=f"du_{s}")[:, :bw]
            nc.vector.tensor_mul(du_c[:], delta_c[:], xc_a[:])

            state[("xca", s)] = xc_a
            state[("sz", s)] = sz
            state[("bbcc", s)] = bbcc
            state[("delta", s)] = delta_c
            state[("du", s)] = du_c

        def _scan(s):
            cs, bw = BLOCKS[s]
            ce = cs + bw
            nhh = bw // T
            xc_a = state.pop(("xca", s))
            sz = state.pop(("sz", s))
            bbcc = state.pop(("bbcc", s))
            delta_c = state.pop(("delta", s))
            du_c = state.pop(("du", s))

            # GPSIMD runs in-order: issue all its dBu products up front so it
            # never stalls behind a tmp waiting on a scan.
            gp_dBu = {}
            for n in range(NGP, NST):
                dBu = lp.tile([C, T2], bf16, tag="dBug",
                              name=f"dBu_{s}_{n}")[:, :bw]
                nc.gpsimd.tensor_mul(dBu[:], du_c[:], bbcc[n][:, 0, :])
                gp_dBu[n] = dBu

            yPh = [psy.tile([C, T], f32, tag="psy", name=f"yP_{s}_{hh}")
                   for hh in range(nhh)]
            for n in range(NST):
                dA = lp.tile([C, T2], f32, tag="dA", name=f"dA_{s}_{n}")[:, :bw]
                nc.scalar.activation(dA[:], delta_c[:], AF.Exp,
                                     scale=aneg_sb[:, n:n + 1])
                if n < NGP:
                    dBu = lp.tile([C, T2], bf16, tag="dBu",
                                  name=f"dBu_{s}_{n}")[:, :bw]
                    nc.vector.tensor_mul(dBu[:], du_c[:], bbcc[n][:, 0, :])
                else:
                    dBu = gp_dBu[n]

                h = hp.tile([C, T2], bf16, tag="h", name=f"h_{s}_{n}")[:, :bw]
                init = 0.0 if s == 0 else carry[:, n:n + 1]
                nc.vector.tensor_tensor_scan(h[:], dA[:], dBu[:], init,
                                             OP.mult, OP.add)
                # carry only feeds the next block's scan: keep it off the
                # vector engine's critical stream
                nc.gpsimd.tensor_copy(carry[:, n:n + 1], h[:, bw - 1:bw])

                tmp = lp.tile([C, T2], bf16, tag="tmp",
                              name=f"tmp_{s}_{n}")[:, :bw]
                teng = nc.gpsimd if n in TMP_GP else nc.vector
                teng.tensor_mul(tmp[:], h[:], bbcc[n][:, 1, :])
                for hh in range(nhh):
                    nc.tensor.matmul(yPh[hh][:], ident_sb[:],
                                     tmp[:, hh * T:(hh + 1) * T],
                                     start=(n == 0), stop=(n == NST - 1))

            # ---- D*u, gate, out_proj partial ----
            for hh in range(nhh):
                sl = slice(hh * T, (hh + 1) * T)
                yg = tl.tile([C, T], f32, tag="yg", name=f"yg_{s}_{hh}")
                nc.vector.scalar_tensor_tensor(yg[:], xc_a[:, sl],
                                               dvec_sb[:, 0:1], yPh[hh][:],
                                               OP.mult, OP.add)
                y2 = tl.tile([C, T], bf16, tag="y2", name=f"y2_{s}_{hh}")
                nc.gpsimd.tensor_mul(y2[:], yg[:], sz[:, sl])
                outP = pmix.tile([C, T], f32, tag="pmix", name=f"outP_{s}_{hh}")
                nc.tensor.matmul(outP[:], wout_sb[:], y2[:])
                osb = tl.tile([C, T], f32, tag="osb", name=f"osb_{s}_{hh}")
                nc.scalar.activation(osb[:], outP[:], AF.Copy)
                nc.sync.dma_start(
                    dram["out_part"][:, cs + hh * T: cs + (hh + 1) * T], osb[:])

        # Software pipeline over variable-width blocks, depth 2: the 1x1
        # projection runs at block b (3x3 conv needs a one-row halo), the
        # rest of the front end at b-1, the scan stage at b-2. Smaller
        # leading blocks shorten the pipeline fill.
        NBK = len(BLOCKS)
        for b in range(NBK + 2):
            if b < NBK:
                cs, bw = BLOCKS[b]
                nrow = bw // WW
                xck = pxck.tile([C, T2], bf16, tag="xck", name=f"xck_{b}")[:, :bw]
                nc.sync.dma_start(xck[:], dram["x"][:, cs:cs + bw])
                for g in range(2):
                    ps = fps.tile([C, T2], f32, tag="fps",
                                  name=f"proj_{b}_{g}")[:, :bw]
                    mmacc(ps, [(wp_sb[:, g * C:(g + 1) * C], xck[:])])
                    row0 = cs // WW
                    dst = xp1[g][:, G + (row0 + 1) * PW + 1:
                                 G + (row0 + nrow + 1) * PW + 1]
                    dst = dst.rearrange("p (r w) -> p r w", w=PW)[:, :, 0:WW]
                    nc.scalar.activation(dst,
                                         ps.rearrange("p (r w) -> p r w", w=WW),
                                         AF.Copy)
            if 1 <= b <= NBK:
                _front(b - 1)
            if b >= 2:
                _scan(b - 2)


def _build_program():
    from concourse import bacc, tile, mybir

    nc = bacc.Bacc("TRN2", target_bir_lowering=False, debug=False, num_devices=8)
    f32 = mybir.dt.float32
    bf16 = mybir.dt.bfloat16

    def din(name, shape, dtype=f32):
        return nc.dram_tensor(name, shape, dtype, kind="ExternalInput").ap()

    dram = {
        "x": din("x", (C, L), bf16),
        "wp": din("wp", (C, DI), bf16),
        "wd": din("wd", (9, 2, C, C), bf16),
        "win": din("win", (C, 3 * C), bf16),
        "w1d": din("w1d", (2, 4, C, C), bf16),
        "b1d": din("b1d", (C, 2)),
        "wxp": din("wxp", (2, C, 38), bf16),
        "wdt": din("wdt", (32, C), bf16),
        "bdt": din("bdt", (C, 1)),
        "dvec": din("dvec", (C, 1)),
        "wout": din("wout", (C, C), bf16),
        "aneg": din("aneg", (C, NST)),
        "ident": din("ident", (C, C), bf16),
        "out_part": nc.dram_tensor("out_part", (C, L), f32,
                                   kind="ExternalOutput").ap(),
    }

    with tile.TileContext(nc) as tc:
        _emit(tc, nc, mybir, dram)
    nc.compile()
    return nc


def get_program():
    if "nc" not in _CACHE:
        _CACHE["nc"] = _build_program()
    return _CACHE["nc"]


def make_core_inputs(inputs, b, half):
    import ml_dtypes

    bf = ml_dtypes.bfloat16
    perm = np.concatenate([
        np.arange(half * C, half * C + C),
        np.arange((1 - half) * C, (1 - half) * C + C),
    ])
    loc = perm[:C]

    a = np.exp(np.asarray(inputs["A_log"], np.float64))[loc].astype(np.float32)

    wd = np.empty((9, 2, C, C), np.float32)
    dw = np.asarray(inputs["dconv_w"], np.float32)   # (96, 192, 3, 3)
    for tap in range(9):
        dy, dx = tap // 3, tap % 3
        for g in range(2):
            wd[tap, g] = dw[:, g * C:(g + 1) * C, dy, dx].T

    w_in = np.asarray(inputs["in_proj_w"], np.float32)
    win = np.concatenate([w_in[perm[:C]].T, w_in[perm[C:]].T,
                          w_in[DI + loc].T], axis=1)

    w1 = np.asarray(inputs["conv1d_w"], np.float32)[perm]   # (192, 4)
    w1d = np.zeros((2, 4, C, C), np.float32)
    for g in range(2):
        for k in range(4):
            np.fill_diagonal(w1d[g, k], w1[g * C:(g + 1) * C, k])
    b1 = np.asarray(inputs["conv1d_b"], np.float32)[perm]
    b1d = np.stack([b1[:C], b1[C:]], axis=1)

    wxp_full = np.asarray(inputs["x_proj_w"], np.float32)[:, perm]  # (38, 192)
    wxp = np.stack([wxp_full[:, :C].T, wxp_full[:, C:].T], axis=0)

    wdt = np.zeros((32, C), np.float32)
    wdt[:DTR] = np.asarray(inputs["dt_proj_w"], np.float32)[loc].T

    return {
        "x": np.ascontiguousarray(
            np.asarray(inputs["x"], np.float32)[b].reshape(C, L)).astype(bf),
        "wp": np.ascontiguousarray(
            np.asarray(inputs["proj_w"], np.float32)[:, :, 0, 0].T).astype(bf),
        "wd": wd.astype(bf),
        "win": np.ascontiguousarray(win).astype(bf),
        "w1d": w1d.astype(bf),
        "b1d": np.ascontiguousarray(b1d),
        "wxp": np.ascontiguousarray(wxp).astype(bf),
        "wdt": wdt.astype(bf),
        "bdt": np.asarray(inputs["dt_proj_b"], np.float32)[loc, None],
        "dvec": np.asarray(inputs["D"], np.float32)[loc, None],
        "wout": np.ascontiguousarray(
            np.asarray(inputs["out_proj_w"], np.float32)[:, loc].T).astype(bf),
        "aneg": -a,
        "ident": np.eye(C, dtype=np.float32).astype(bf),
    }


def kernel(**inputs):
    from concourse import bass_utils

    nc = get_program()
    in_maps = [make_core_inputs(inputs, b, half)
               for b in range(4) for half in range(2)]
    res = bass_utils.run_bass_kernel_spmd(nc, in_maps, core_ids=list(range(8)))
    out = np.zeros((4, C, L), np.float32)
    for b in range(4):
        out[b] = res.results[2 * b]["out_part"] + res.results[2 * b + 1]["out_part"]
    return out.reshape(4, C, HH, WW)



# revision 9
# speedup vs baseline: 1.1370x; 1.1370x over previous
"""Trainium2 Bass kernel for CSSM (Mamba-style 2D selective scan block).

Sharding: 8 cores = 4 batch x 2 d_inner-halves. Each core computes the
full front-end for its batch element, the selective scan for its 96
d_inner channels x 16 states, and a partial output projection. The host
sums the two partial outputs per batch element.

Key structure:
- The 1x1 input projection is folded into the 3x3 depthwise-ish conv on
  the host (W_eff[o,c,tap] = sum_i dconv_w[o,i,tap] proj_w[i,c]), so the
  conv needs 9 matmuls per PSUM unit instead of 18 and the proj
  stage disappears (PE matmul cost is K-independent).
- The causal depthwise conv1d is folded into in_proj the same way
  (4-tap full conv applied straight to the conv trunk).
- dA_n = exp(-(n+1) delta): the first NACT states come from the scalar
  engine (exp, bf16 out); the rest are bf16 products dA_{NACT-1} *
  dA_{n-NACT} on the vector engine (2x mode).
- Scans (tensor_tensor_scan, always 1x) are split between the DVE and
  GPSIMD(Pool) engines; dBu/tmp products stay on DVE where bf16 runs 2x.
- D*u enters the y accumulation as a diag(D) matmul into the same PSUM
  banks as the per-state ident matmuls; the gate multiply reads PSUM
  directly on gpsimd.
- Scalar-engine ops are grouped so the activation table switches only
  twice per block (Silu group vs Exp/Ln group).
"""
import sys

sys.path.insert(0, "/opt/trn_rl_repo")

import numpy as np

C = 96            # d_model; also channels per d_inner half
DI = 192          # d_inner
NST = 16          # d_state
DTR = 6           # dt_rank
HH = 64
WW = 64
L = HH * WW       # 4096
T = 512           # matmul moving-dim chunk
T2 = 1024         # block chunk
PW = WW + 2       # 66: padded row width for the 3x3 conv
G = 68            # left guard of the padded conv buffer
PADLEN = G + (HH + 2) * PW + 68
BLOCKS = ((0, 1024), (1024, 1024), (2048, 1024), (3072, 1024))
NACT = 16         # states 0..NACT-1: dA by scalar-engine exp (rest: DVE muls)
POOL_DBU = frozenset(range(4, 16))   # states whose dBu mul runs on gpsimd

_CACHE = {}


def _emit(tc, nc, mybir, dram):
    from contextlib import ExitStack

    from concourse import bass

    f32 = mybir.dt.float32
    bf16 = mybir.dt.bfloat16
    AF = mybir.ActivationFunctionType
    OP = mybir.AluOpType

    def mmacc(out, pairs, start=True, stop=True, ncols=None):
        """Matmul with free-dim split into even <=512-col PSUM-bank groups."""
        n = ncols if ncols is not None else out.shape[-1]
        nchunk = -(-n // T)
        step = -(-n // nchunk)
        bounds = list(range(0, n, step)) + [n]
        for c0, c1 in zip(bounds[:-1], bounds[1:]):
            for i, (lh, rh) in enumerate(pairs):
                nc.tensor.matmul(out[:, c0:c1], lh, rh[:, c0:c1],
                                 start=start and i == 0,
                                 stop=stop and i == len(pairs) - 1)

    with ExitStack() as ctx:
        ec = ctx.enter_context
        consts = ec(tc.tile_pool(name="consts", bufs=1))
        persist = ec(tc.tile_pool(name="persist", bufs=1))
        dpool = ec(tc.tile_pool(name="dpool", bufs=1, space="DRAM"))
        fw = ec(tc.tile_pool(name="fw", bufs=1))
        pxc2 = ec(tc.tile_pool(name="pxc2", bufs=2))
        pxca = ec(tc.tile_pool(name="pxca", bufs=3))
        pxcb = ec(tc.tile_pool(name="pxcb", bufs=2))
        psz = ec(tc.tile_pool(name="psz", bufs=2))
        pxd = ec(tc.tile_pool(name="pxd", bufs=2))
        ped = ec(tc.tile_pool(name="ped", bufs=2))
        dl = ec(tc.tile_pool(name="dl", bufs=2))
        dap = ec(tc.tile_pool(name="dap", bufs=max(3, NST - NACT + 3)))
        lp = ec(tc.tile_pool(name="lp", bufs=3))
        gbp = ec(tc.tile_pool(name="gbp", bufs=len(POOL_DBU) + 2))
        hp = ec(tc.tile_pool(name="hp", bufs=3))
        bc = ec(tc.tile_pool(name="bc", bufs=18))
        tl = ec(tc.tile_pool(name="tl", bufs=2))
        pbig = ec(tc.tile_pool(name="pbig", bufs=2, space="PSUM"))
        pbank = ec(tc.tile_pool(name="pbank", bufs=2, space="PSUM"))
        psy = ec(tc.tile_pool(name="psy", bufs=2, space="PSUM"))

        def cload(name, shape, dtype=f32, rearr=None, pool=None):
            t = (pool or consts).tile(list(shape), dtype, tag=name)
            src = dram[name]
            if rearr is not None:
                src = src.rearrange(rearr)
            nc.sync.dma_start(t[:], src)
            return t

        wdt_sb = cload("wdt", (32, C), bf16)
        bdt_sb = cload("bdt", (C, 1))
        wout_sb = cload("wout", (C, C), bf16)
        b1d_sb = cload("b1d", (C, 2))
        aneg_sb = cload("aneg", (C, NST))       # -exp(A_log), local rows
        ident_sb = cload("ident", (C, C), bf16)
        ddiag_sb = cload("ddiag", (C, C), bf16)
        wde_sb = cload("wde", (C, 9, C), bf16, "t k m -> k t m", pool=fw)
        w1e_sb = cload("w1e", (C, 8, C), bf16, "g t k m -> k (g t) m", pool=fw)
        winz_sb = cload("winz", (C, C), bf16, pool=fw)
        wxp_sb = cload("wxp", (C, 2, 38), bf16, "g k m -> k g m", pool=fw)

        carry = persist.tile([C, NST], f32, tag="carry")
        xdd = dpool.tile([38, L], bf16, tag="xdd")

        xp1 = persist.tile([C, PADLEN], bf16, tag="xp1")
        nc.gpsimd.memset(xp1[:], 0.0)

        state = {"xc2_prev": None}

        def _front(s):
            cs, bw = BLOCKS[s]
            ce = cs + bw
            nhh = bw // T
            # ---- 3x3 conv (proj folded in); <=7-row units, one bank each ---
            nrow = bw // WW
            units = []
            ro = 0
            while ro < nrow:
                units.append((ro, min(7, nrow - ro)))
                ro += 7
            xc2 = pxc2.tile([C, 3 + T2], bf16, tag="xc2", name=f"xc2_{s}")
            if s == 0:
                nc.gpsimd.memset(xc2[:, 0:3], 0.0)
            else:
                pw = BLOCKS[s - 1][1]
                nc.gpsimd.tensor_copy(xc2[:, 0:3],
                                      state["xc2_prev"][:, pw:pw + 3])
            for u, (ro, rows) in enumerate(units):
                r0 = cs // WW + ro
                cols = rows * PW
                base = G + (r0 + 1) * PW
                psd = pbank.tile([C, T], f32, tag="pbank", name=f"dconv_{s}_{u}")
                pairs = []
                for tap in range(9):
                    dy, dx = tap // 3, tap % 3
                    shift = (dy - 1) * PW + (dx - 1)
                    pairs.append((wde_sb[:, tap, :],
                                  xp1[:, base + shift: base + shift + cols]))
                mmacc(psd[:, :cols], pairs, ncols=cols)
                srcv = psd[:, :cols].rearrange("p (r w) -> p r w", w=PW)[:, :, 1:65]
                dstv = xc2[:, 3 + ro * WW: 3 + (ro + rows) * WW]
                nc.scalar.activation(dstv.rearrange("p (r w) -> p r w", w=WW),
                                     srcv, AF.Copy)
            state["xc2_prev"] = xc2

            # ---- in_proj+conv1d folded: 4-tap conv on trunk + silu --------
            xc_a = pxca.tile([C, T2], bf16, tag="xc_a", name=f"xc_a_{s}")[:, :bw]
            xc_b = pxcb.tile([C, T2], bf16, tag="xc_b", name=f"xc_b_{s}")[:, :bw]
            for g in range(2):
                ps = pbig.tile([C, T2], f32, tag="pbig", name=f"c1d_{s}_{g}")[:, :bw]
                mmacc(ps, [(w1e_sb[:, g * 4 + k, :], xc2[:, k:k + bw])
                           for k in range(4)])
                nc.scalar.activation(xc_a[:] if g == 0 else xc_b[:], ps[:],
                                     AF.Silu, bias=b1d_sb[:, g:g + 1])
            # ---- z gate ---------------------------------------------------
            sz = psz.tile([C, T2], bf16, tag="sz", name=f"sz_{s}")[:, :bw]
            psgz = pbig.tile([C, T2], f32, tag="pbig", name=f"z_{s}")[:, :bw]
            mmacc(psgz, [(winz_sb[:], xc2[:, 3:3 + bw])])
            nc.scalar.activation(sz[:], psgz[:], AF.Silu)

            # ---- x_proj -> x_dbl block, staged to DRAM --------------------
            psx = pbig.tile([38, T2], f32, tag="pbig", name=f"xp_{s}")[:, :bw]
            mmacc(psx, [(wxp_sb[:, 0, :], xc_a[:]), (wxp_sb[:, 1, :], xc_b[:])])
            x_dbl = pxd.tile([38, T2], bf16, tag="x_dbl", name=f"x_dbl_{s}")[:, :bw]
            nc.scalar.activation(x_dbl[:], psx[:], AF.Copy)
            nc.sync.dma_start(xdd[:, cs:ce], x_dbl[:])

            # ---- B/C broadcasts: ready before the scan stage --------------
            bbcc = []
            for n in range(NST):
                t = bc.tile([C, 2, T2], bf16, tag="bc", name=f"bc_{s}_{n}")[:, :, :bw]
                row = xdd[DTR + n: DTR + n + 1, cs:ce]
                srcb = bass.AP(tensor=row.tensor, offset=row.offset,
                               ap=[[0, C], [NST * L, 2], [1, bw]])
                nc.sync.dma_start(t[:], srcb)
                bbcc.append(t)

            # ---- delta (softplus, bf16) + du ------------------------------
            edt = ped.tile([C, T2], f32, tag="edt", name=f"edt_{s}")[:, :bw]
            for hh in range(nhh):
                psD = pbank.tile([C, T], f32, tag="pbank", name=f"psD_{s}_{hh}")
                nc.tensor.matmul(psD[:], wdt_sb[:],
                                 x_dbl[:32, hh * T:(hh + 1) * T])
                nc.scalar.activation(edt[:, hh * T:(hh + 1) * T], psD[:],
                                     AF.Exp, bias=bdt_sb[:])
            delta_c = dl.tile([C, T2], bf16, tag="delta", name=f"delta_{s}")[:, :bw]
            nc.scalar.activation(delta_c[:], edt[:], AF.Ln, bias=1.0)
            du_c = dl.tile([C, T2], bf16, tag="du", name=f"du_{s}")[:, :bw]
            nc.vector.tensor_mul(du_c[:], delta_c[:], xc_a[:])

            state[("xca", s)] = xc_a
            state[("sz", s)] = sz
            state[("bbcc", s)] = bbcc
            state[("delta", s)] = delta_c
            state[("du", s)] = du_c

        def _scan(s):
            cs, bw = BLOCKS[s]
            nhh = bw // T
            xc_a = state.pop(("xca", s))
            sz = state.pop(("sz", s))
            bbcc = state.pop(("bbcc", s))
            delta_c = state.pop(("delta", s))
            du_c = state.pop(("du", s))

            yPh = [psy.tile([C, T], f32, tag="psy", name=f"yP_{s}_{hh}")
                   for hh in range(nhh)]
            # gpsimd runs in-order: issue all its dBu products up front so it
            # never stalls behind anything waiting on a scan.
            gp_dBu = {}
            for n in sorted(POOL_DBU):
                dBu = gbp.tile([C, T2], bf16, tag="dBug",
                               name=f"dBu_{s}_{n}")[:, :bw]
                nc.gpsimd.tensor_mul(dBu[:], du_c[:], bbcc[n][:, 0, :])
                gp_dBu[n] = dBu

            dAs = []
            for n in range(NST):
                dA = dap.tile([C, T2], bf16, tag="dA", name=f"dA_{s}_{n}")[:, :bw]
                if n < NACT:
                    nc.scalar.activation(dA[:], delta_c[:], AF.Exp,
                                         scale=aneg_sb[:, n:n + 1])
                else:
                    # exp(-(n+1)d) = exp(-NACT d) * exp(-(n+1-NACT) d)
                    nc.vector.tensor_mul(dA[:], dAs[NACT - 1][:],
                                         dAs[n - NACT][:])
                dAs.append(dA)

                if n in POOL_DBU:
                    dBu = gp_dBu[n]
                else:
                    dBu = lp.tile([C, T2], bf16, tag="dBu",
                                  name=f"dBu_{s}_{n}")[:, :bw]
                    nc.vector.tensor_mul(dBu[:], du_c[:], bbcc[n][:, 0, :])

                h = hp.tile([C, T2], bf16, tag="h", name=f"h_{s}_{n}")[:, :bw]
                init = 0.0 if s == 0 else carry[:, n:n + 1]
                nc.vector.tensor_tensor_scan(h[:], dA[:], dBu[:], init,
                                             OP.mult, OP.add)
                nc.gpsimd.tensor_copy(carry[:, n:n + 1], h[:, bw - 1:bw])

                tmp = lp.tile([C, T2], bf16, tag="tmp",
                              name=f"tmp_{s}_{n}")[:, :bw]
                nc.vector.tensor_mul(tmp[:], h[:], bbcc[n][:, 1, :])
                for hh in range(nhh):
                    nc.tensor.matmul(yPh[hh][:], ident_sb[:],
                                     tmp[:, hh * T:(hh + 1) * T],
                                     start=(n == 0), stop=False)

            # ---- D*u into the same accumulators, gate, out_proj -----------
            for hh in range(nhh):
                sl = slice(hh * T, (hh + 1) * T)
                nc.tensor.matmul(yPh[hh][:], ddiag_sb[:], xc_a[:, sl],
                                 start=False, stop=True)
                y2 = tl.tile([C, T], bf16, tag="y2", name=f"y2_{s}_{hh}")
                nc.vector.tensor_mul(y2[:], yPh[hh][:], sz[:, sl])
                outP = pbank.tile([C, T], f32, tag="pbank", name=f"outP_{s}_{hh}")
                nc.tensor.matmul(outP[:], wout_sb[:], y2[:])
                osb = tl.tile([C, T], f32, tag="osb", name=f"osb_{s}_{hh}")
                nc.scalar.activation(osb[:], outP[:], AF.Copy)
                nc.sync.dma_start(
                    dram["out_part"][:, cs + hh * T: cs + (hh + 1) * T], osb[:])

        # Software pipeline, depth 2: x-block DMA at b (3x3 conv needs a
        # one-row halo), the front end at b-1, the scan stage at b-2.
        NBK = len(BLOCKS)
        for b in range(NBK + 2):
            if b < NBK:
                cs, bw = BLOCKS[b]
                nrow = bw // WW
                row0 = cs // WW
                dst = xp1[:, G + (row0 + 1) * PW + 1:
                          G + (row0 + nrow + 1) * PW + 1]
                dst = dst.rearrange("p (r w) -> p r w", w=PW)[:, :, 0:WW]
                nc.sync.dma_start(dst, dram["x"][:, cs:cs + bw]
                                  .rearrange("p (r w) -> p r w", w=WW))
            if 1 <= b <= NBK:
                _front(b - 1)
            if b >= 2:
                _scan(b - 2)


def _build_program():
    from concourse import bacc, tile, mybir

    nc = bacc.Bacc("TRN2", target_bir_lowering=False, debug=False, num_devices=8)
    f32 = mybir.dt.float32
    bf16 = mybir.dt.bfloat16

    def din(name, shape, dtype=f32):
        return nc.dram_tensor(name, shape, dtype, kind="ExternalInput").ap()

    dram = {
        "x": din("x", (C, L), bf16),
        "wde": din("wde", (9, C, C), bf16),
        "w1e": din("w1e", (2, 4, C, C), bf16),
        "winz": din("winz", (C, C), bf16),
        "b1d": din("b1d", (C, 2)),
        "wxp": din("wxp", (2, C, 38), bf16),
        "wdt": din("wdt", (32, C), bf16),
        "bdt": din("bdt", (C, 1)),
        "wout": din("wout", (C, C), bf16),
        "aneg": din("aneg", (C, NST)),
        "ident": din("ident", (C, C), bf16),
        "ddiag": din("ddiag", (C, C), bf16),
        "out_part": nc.dram_tensor("out_part", (C, L), f32,
                                   kind="ExternalOutput").ap(),
    }

    with tile.TileContext(nc) as tc:
        _emit(tc, nc, mybir, dram)
    nc.compile()
    return nc


def get_program():
    if "nc" not in _CACHE:
        _CACHE["nc"] = _build_program()
    return _CACHE["nc"]


def make_core_inputs(inputs, b, half):
    import ml_dtypes

    bf = ml_dtypes.bfloat16
    perm = np.concatenate([
        np.arange(half * C, half * C + C),
        np.arange((1 - half) * C, (1 - half) * C + C),
    ])
    loc = perm[:C]

    a = np.exp(np.asarray(inputs["A_log"], np.float64))[loc].astype(np.float32)

    # fold 1x1 proj into the 3x3 conv: W_eff[o,c,tap] = sum_i dw[o,i,t] wp[i,c]
    dw = np.asarray(inputs["dconv_w"], np.float32)   # (96, 192, 3, 3)
    wp = np.asarray(inputs["proj_w"], np.float32)[:, :, 0, 0]  # (192, 96)
    wde = np.empty((9, C, C), np.float32)            # (tap, c_in, o)
    for tap in range(9):
        dy, dx = tap // 3, tap % 3
        wde[tap] = (dw[:, :, dy, dx] @ wp).T

    # fold conv1d into in_proj: W1eff[g,k][c,d] = win_g[d,c] * w1[g*96+d,k]
    w_in = np.asarray(inputs["in_proj_w"], np.float32)
    w1 = np.asarray(inputs["conv1d_w"], np.float32)[perm]   # (192, 4)
    w1e = np.empty((2, 4, C, C), np.float32)
    for g in range(2):
        wing = w_in[perm[g * C:(g + 1) * C]]         # (96 d, 96 c)
        for k in range(4):
            w1e[g, k] = (wing * w1[g * C:(g + 1) * C, k][:, None]).T
    winz = w_in[DI + loc].T                          # (96 c, 96 d)

    b1 = np.asarray(inputs["conv1d_b"], np.float32)[perm]
    b1d = np.stack([b1[:C], b1[C:]], axis=1)

    wxp_full = np.asarray(inputs["x_proj_w"], np.float32)[:, perm]  # (38, 192)
    wxp = np.stack([wxp_full[:, :C].T, wxp_full[:, C:].T], axis=0)

    wdt = np.zeros((32, C), np.float32)
    wdt[:DTR] = np.asarray(inputs["dt_proj_w"], np.float32)[loc].T

    return {
        "x": np.ascontiguousarray(
            np.asarray(inputs["x"], np.float32)[b].reshape(C, L)).astype(bf),
        "wde": np.ascontiguousarray(wde).astype(bf),
        "w1e": np.ascontiguousarray(w1e).astype(bf),
        "winz": np.ascontiguousarray(winz).astype(bf),
        "b1d": np.ascontiguousarray(b1d),
        "wxp": np.ascontiguousarray(wxp).astype(bf),
        "wdt": wdt.astype(bf),
        "bdt": np.asarray(inputs["dt_proj_b"], np.float32)[loc, None],
        "wout": np.ascontiguousarray(
            np.asarray(inputs["out_proj_w"], np.float32)[:, loc].T).astype(bf),
        "aneg": -a,
        "ident": np.eye(C, dtype=np.float32).astype(bf),
        "ddiag": np.diag(np.asarray(inputs["D"], np.float32)[loc]).astype(bf),
    }


def kernel(**inputs):
    from concourse import bass_utils

    nc = get_program()
    in_maps = [make_core_inputs(inputs, b, half)
               for b in range(4) for half in range(2)]
    res = bass_utils.run_bass_kernel_spmd(nc, in_maps, core_ids=list(range(8)))
    out = np.zeros((4, C, L), np.float32)
    for b in range(4):
        out[b] = res.results[2 * b]["out_part"] + res.results[2 * b + 1]["out_part"]
    return out.reshape(4, C, HH, WW)
